# revision 10
# baseline (speedup 1.0000x reference)
"""Trainium2 Bass kernel for AliceAttention (dense transformer attention layer).

Reference computation (fp32):
    q/k/v = hidden @ W{q,k,v}.T  -> [B,S,NH,HD], RoPE(q,k),
    scores = q k^T / sqrt(HD) + mask, softmax, out = attn @ v,
    y = out @ Wo.T

Sharding: tensor-parallel over the 32 heads -> 4 heads per core across 8
NeuronCores. Each core computes q/k/v for its heads (columns of the
projections), full attention for its 8 (batch, head) pairs, and a partial
o_proj ( y_c = ao_c @ Wo[:, cols_c].T ); the 8 fp32 partials are summed on
the host.

Device layout choices:
  * All big matmuls run in bf16 (PE streams 1 column/cycle; fp32 is 4x
    slower). PSUM accumulation is fp32.
  * q,k are produced directly in transposed layout qT/kT = [d, t] by using
    W.T slices as the stationary operand. RoPE's rotate_half becomes a
    [128,128] +/-1 permutation matmul (P @ qT) plus elementwise combines.
  * Scores are computed transposed, scores_T = [t_k, t_q] , so that
    (a) attn @ v needs no transposes: outT[d, t_q] = v[t_k, d].T @ exp_T,
    (b) softmax denominators are a ones-column matmul over the partition
        axis, accumulated in PSUM alongside the AV matmul.
    Normalisation is deferred to after AV: outT *= (1/sums) broadcast
    across partitions via a K=1 ones matmul (float32r, exact-ish).
  * Causal masking: strictly-masked [t_k, t_q] tiles are skipped entirely;
    diagonal tiles add one of 4 precomputed [128,512] mask patterns. A
    general (non-causal) additive mask falls back to streaming mask tiles
    for every block; an all-zero mask skips masking but computes all
    blocks.
"""

import sys

import numpy as np
import ml_dtypes
from contextlib import ExitStack

import orjson

import concourse.bass as bass
import concourse.mybir as mybir
import concourse.tile as tile
import concourse.bass2jax as bass2jax
from concourse.bass_utils import run_bass_kernel_spmd

# ─────────────────────────────────────────────────────────────────────────
# This container's walrus rejects instructions carrying more semaphore
# waits than their ISA struct can hold (e.g. the Tile tail-drain with 5).
# Split excess waits into preceding wait-only EventSemaphore instructions
# (2 waits each) on the same engine — semantically identical.
# ─────────────────────────────────────────────────────────────────────────
_WAIT_CAP = {"EventSemaphore": 2}
_DEFAULT_WAIT_CAP = 1


def _legalize_bir_waits(bir_bytes: bytes) -> bytes:
    d = orjson.loads(bir_bytes)
    changed = False
    for fn in d.get("functions", []):
        for blk in fn.get("blocks", []):
            insts = blk.get("instructions")
            if not insts:
                continue
            out = []
            for inst in insts:
                si = inst.get("sync_info")
                waits = (si or {}).get("on_wait") or []
                cap = _WAIT_CAP.get(inst.get("opcode"), _DEFAULT_WAIT_CAP)
                if len(waits) > cap:
                    excess, keep = waits[:-cap], waits[-cap:]
                    for i in range(0, len(excess), 2):
                        out.append(
                            {
                                "debug": inst.get("debug"),
                                "engine": inst["engine"],
                                "ins": [],
                                "outs": [],
                                "name": f"{inst['name']}_xw{i}",
                                "opcode": "EventSemaphore",
                                "sync_info": {
                                    "on_update": [],
                                    "on_wait": excess[i : i + 2],
                                },
                            }
                        )
                    si["on_wait"] = keep
                    changed = True
                out.append(inst)
            blk["instructions"] = out
    return orjson.dumps(d) if changed else bir_bytes


if not getattr(bass2jax, "_wait_legalize_patched", False):
    _orig_compile_bir_kernel = bass2jax.compile_bir_kernel

    def _patched_compile_bir_kernel(ant_bir_str, compile_dir_path, **kw):
        return _orig_compile_bir_kernel(
            _legalize_bir_waits(ant_bir_str), compile_dir_path, **kw
        )

    bass2jax.compile_bir_kernel = _patched_compile_bir_kernel
    bass2jax._wait_legalize_patched = True

# ─────────────────────────────────────────────────────────────────────────
# Problem constants (hardcoded per contract)
# ─────────────────────────────────────────────────────────────────────────
B, S, H, NH, HD = 2, 2048, 4096, 32, 128
THETA = 10000.0
NCORES = 8
HPC = NH // NCORES          # heads per core = 4
OC = HPC * HD               # output cols per core = 512
T = B * S                   # 4096 tokens
KT = H // 128               # 32 contraction tiles for projections
TB = 512                    # t-block width in phase A
NTB = T // TB               # 8 t-blocks
NQ = S // 512               # 4 query blocks per pair
NK = S // 128               # 16 key tiles per pair
SCALE = 1.0 / float(np.sqrt(HD))

F32 = mybir.dt.float32
F32R = mybir.dt.float32r
BF16 = mybir.dt.bfloat16
BF = ml_dtypes.bfloat16
EXPF = mybir.ActivationFunctionType.Exp


def _build(mode: str) -> bass.Bass:
    """mode: 'causal' (skip masked tiles, 4 diag patterns),
    'zeros' (no mask, all tiles), 'general' (stream fp32 mask tiles)."""
    nc = bass.Bass()

    xt = nc.declare_dram_parameter("xt", [H, T], BF16, isOutput=False)
    wq = nc.declare_dram_parameter("wq", [H, OC], BF16, isOutput=False)
    wk = nc.declare_dram_parameter("wk", [H, OC], BF16, isOutput=False)
    wv = nc.declare_dram_parameter("wv", [H, OC], BF16, isOutput=False)
    wo = nc.declare_dram_parameter("wo", [OC, H], BF16, isOutput=False)
    cost = nc.declare_dram_parameter("cost", [HD, T], F32, isOutput=False)
    sint = nc.declare_dram_parameter("sint", [HD, T], F32, isOutput=False)
    pt = nc.declare_dram_parameter("pt", [HD, HD], BF16, isOutput=False)
    ones_bf = nc.declare_dram_parameter("ones_bf", [128, 1], BF16, isOutput=False)
    ones_f = nc.declare_dram_parameter("ones_f", [1, 128], F32, isOutput=False)
    ones_cf = nc.declare_dram_parameter("ones_cf", [128, 1], F32, isOutput=False)
    if mode == "causal":
        mdiag = nc.declare_dram_parameter("mdiag", [4 * 128, 512], BF16, isOutput=False)
    elif mode == "general":
        maskt = nc.declare_dram_parameter("maskt", [S, S], F32, isOutput=False)
    y = nc.declare_dram_parameter("y", [T, H], F32, isOutput=True)

    # DRAM scratch (per core): roped qT/kT [OC, T] (f32r) and v [T, OC] (bf16)
    qts = nc.dram_tensor("qts", [OC, T], BF16)
    kts = nc.dram_tensor("kts", [OC, T], BF16)
    vs = nc.dram_tensor("vs", [T, OC], BF16)

    with tile.TileContext(nc) as tc, ExitStack() as octx:
        # ── pools that live for the whole kernel ──
        const_pool = octx.enter_context(tc.tile_pool(name="const", bufs=1))

        pt_sb = const_pool.tile([HD, HD], BF16)
        nc.sync.dma_start(out=pt_sb[:], in_=pt[:])
        ones_sb = const_pool.tile([128, 1], BF16)
        nc.sync.dma_start(out=ones_sb[:], in_=ones_bf[:])
        onesf_sb = const_pool.tile([1, 128], F32)
        nc.sync.dma_start(out=onesf_sb[:], in_=ones_f[:])
        onesr_sb = const_pool.tile([1, 128], F32R)
        nc.vector.tensor_copy(onesr_sb[:], onesf_sb[:])
        onescf_sb = const_pool.tile([128, 1], F32)
        nc.sync.dma_start(out=onescf_sb[:], in_=ones_cf[:])
        onescr_sb = const_pool.tile([128, 1], F32R)
        nc.vector.tensor_copy(onescr_sb[:], onescf_sb[:])
        if mode == "causal":
            md_sb = const_pool.tile([128, 4 * 512], BF16)
            nc.sync.dma_start(
                out=md_sb[:].rearrange("p (r c) -> p r c", r=4),
                in_=mdiag.rearrange("(r p) c -> p r c", p=128),
            )

        # ═════════ Phase A: QKV projections + RoPE, spill to DRAM ═════════
        with ExitStack() as actx:
            w_pool = actx.enter_context(tc.tile_pool(name="wqkv", bufs=1))
            x_pool = actx.enter_context(tc.tile_pool(name="xblk", bufs=2))
            cs_pool = actx.enter_context(tc.tile_pool(name="cosin", bufs=2))
            ev_pool = actx.enter_context(tc.tile_pool(name="evac", bufs=3))
            rp_pool = actx.enter_context(tc.tile_pool(name="rope", bufs=3))
            ps_pool = actx.enter_context(
                tc.tile_pool(name="psA", bufs=3, space="PSUM")
            )
            rot_pool = actx.enter_context(
                tc.tile_pool(name="psRot", bufs=2, space="PSUM")
            )

            # weights resident: [128, k*OC + o] layouts; wq first so the
            # first accumulation can start as early as possible
            wq_sb = w_pool.tile([128, KT * OC], BF16, tag="wq")
            wk_sb = w_pool.tile([128, KT * OC], BF16, tag="wk")
            wv_sb = w_pool.tile([128, KT * OC], BF16, tag="wv")
            nc.sync.dma_start(
                out=wq_sb[:].rearrange("p (k o) -> p k o", k=KT),
                in_=wq.rearrange("(k p) o -> p k o", p=128),
            )

            for tb in range(NTB):
                tsl = slice(tb * TB, (tb + 1) * TB)
                x_sb = x_pool.tile([128, KT * TB], BF16, tag="x")
                nc.sync.dma_start(
                    out=x_sb[:].rearrange("p (k t) -> p k t", k=KT),
                    in_=xt[:, tsl].rearrange("(k p) t -> p k t", p=128),
                )
                cos_sb = cs_pool.tile([HD, TB], F32, tag="cos")
                sin_sb = cs_pool.tile([HD, TB], F32, tag="sin")
                nc.sync.dma_start(out=cos_sb[:], in_=cost[:, tsl])
                nc.sync.dma_start(out=sin_sb[:], in_=sint[:, tsl])
                if tb == 0:
                    for w_dram, w_sb in ((wk, wk_sb), (wv, wv_sb)):
                        nc.sync.dma_start(
                            out=w_sb[:].rearrange("p (k o) -> p k o", k=KT),
                            in_=w_dram.rearrange("(k p) o -> p k o", p=128),
                        )

                # q and k: out tiles [o 128, t 512], RoPE, spill
                for which, w_sb, spill in (("q", wq_sb, qts), ("k", wk_sb, kts)):
                    for ot in range(HPC):
                        ps = ps_pool.tile([128, TB], F32, tag="proj")
                        for k in range(KT):
                            nc.tensor.matmul(
                                ps[:],
                                w_sb[:, k * OC + ot * 128 : k * OC + (ot + 1) * 128],
                                x_sb[:, k * TB : (k + 1) * TB],
                                start=(k == 0),
                                stop=(k == KT - 1),
                            )
                        raw_sb = ev_pool.tile([128, TB], BF16, tag="rawqk")
                        nc.vector.tensor_copy(raw_sb[:], ps[:])
                        rot_ps = rot_pool.tile([128, TB], F32, tag="rot")
                        nc.tensor.matmul(
                            rot_ps[:], pt_sb[:], raw_sb[:], start=True, stop=True
                        )
                        t1 = rp_pool.tile([128, TB], F32, tag="t1")
                        nc.vector.tensor_mul(t1[:], raw_sb[:], cos_sb[:])
                        t2 = rp_pool.tile([128, TB], F32, tag="t2")
                        nc.vector.tensor_mul(t2[:], rot_ps[:], sin_sb[:])
                        roped = ev_pool.tile([128, TB], BF16, tag="roped")
                        nc.vector.tensor_add(roped[:], t1[:], t2[:])
                        nc.sync.dma_start(
                            out=spill[ot * 128 : (ot + 1) * 128, tsl], in_=roped[:]
                        )

                # v: out tiles [t 128, o 512] (natural layout), spill
                for mt in range(TB // 128):
                    ps = ps_pool.tile([128, OC], F32, tag="proj")
                    for k in range(KT):
                        nc.tensor.matmul(
                            ps[:],
                            x_sb[:, k * TB + mt * 128 : k * TB + (mt + 1) * 128],
                            wv_sb[:, k * OC : (k + 1) * OC],
                            start=(k == 0),
                            stop=(k == KT - 1),
                        )
                    v_sb = ev_pool.tile([128, OC], BF16, tag="vout")
                    nc.vector.tensor_copy(v_sb[:], ps[:])
                    nc.sync.dma_start(
                        out=vs[tb * TB + mt * 128 : tb * TB + (mt + 1) * 128, :],
                        in_=v_sb[:],
                    )

        # ── residents for phases B+C (allocated after phase A frees SBUF) ──
        ao_pool = octx.enter_context(tc.tile_pool(name="ao", bufs=1))
        wo_pool = octx.enter_context(tc.tile_pool(name="wo", bufs=1))
        # attention output, transposed: one [128, T] tile per local head
        aoT = [
            ao_pool.tile([HD, T], BF16, tag=f"aoT{hl}", name=f"aoT{hl}")
            for hl in range(HPC)
        ]
        # Wo.T resident: [128, hl*H + hout]
        wo_sb = wo_pool.tile([128, HPC * H], BF16)
        nc.sync.dma_start(
            out=wo_sb[:].rearrange("p (hl n) -> p hl n", hl=HPC),
            in_=wo.rearrange("(hl p) n -> p hl n", p=128),
        )

        # ═════════ Phase B: attention per (batch, local head) ═════════
        with ExitStack() as bctx:
            qk_pool = bctx.enter_context(tc.tile_pool(name="qkv_pair", bufs=2))
            exp_pool = bctx.enter_context(tc.tile_pool(name="exp", bufs=4))
            nrm_pool = bctx.enter_context(tc.tile_pool(name="nrm", bufs=3))
            if mode == "general":
                mt_pool = bctx.enter_context(tc.tile_pool(name="mtile", bufs=4))
            sc_pool = bctx.enter_context(
                tc.tile_pool(name="psSc", bufs=3, space="PSUM")
            )
            av_pool = bctx.enter_context(
                tc.tile_pool(name="psAv", bufs=2, space="PSUM")
            )
            sm_pool = bctx.enter_context(
                tc.tile_pool(name="psSum", bufs=2, space="PSUM")
            )
            bc_pool = bctx.enter_context(
                tc.tile_pool(name="psBc", bufs=1, space="PSUM")
            )

            for b in range(B):
                for hl in range(HPC):
                    qT_sb = qk_pool.tile([HD, S], BF16, tag="qTh")
                    kT_sb = qk_pool.tile([HD, S], BF16, tag="kTh")
                    v_sb = qk_pool.tile([128, NK * HD], BF16, tag="vh")
                    osl = slice(hl * 128, (hl + 1) * 128)
                    bsl = slice(b * S, (b + 1) * S)
                    nc.sync.dma_start(out=qT_sb[:], in_=qts[osl, bsl])
                    nc.sync.dma_start(out=kT_sb[:], in_=kts[osl, bsl])
                    nc.sync.dma_start(
                        out=v_sb[:].rearrange("p (k d) -> p k d", k=NK),
                        in_=vs[bsl, osl].rearrange("(k p) d -> p k d", p=128),
                    )
                    rc_all = nrm_pool.tile([1, NQ * 512], F32R, tag="rcall")

                    for j in range(NQ):
                        if mode == "causal":
                            kept = list(range(min(NK, 4 * j + 4)))
                        else:
                            kept = list(range(NK))
                        qsl = slice(j * 512, (j + 1) * 512)
                        av_ps = av_pool.tile([128, 512], F32, tag="av")
                        sm_ps = sm_pool.tile([1, 512], F32, tag="sm")
                        acc_sb = nrm_pool.tile([128, 512], F32R, tag="acc")
                        for i, ki in enumerate(kept):
                            sc_ps = sc_pool.tile([128, 512], F32, tag="sc")
                            nc.tensor.matmul(
                                sc_ps[:],
                                kT_sb[:, ki * 128 : (ki + 1) * 128],
                                qT_sb[:, qsl],
                                start=True,
                                stop=True,
                            )
                            if mode == "causal" and ki >= 4 * j:
                                r = ki - 4 * j
                                nc.vector.tensor_add(
                                    sc_ps[:],
                                    sc_ps[:],
                                    md_sb[:, r * 512 : (r + 1) * 512],
                                )
                            elif mode == "general":
                                m_sb = mt_pool.tile([128, 512], F32, tag="mt")
                                nc.sync.dma_start(
                                    out=m_sb[:],
                                    in_=maskt[ki * 128 : (ki + 1) * 128, qsl],
                                )
                                nc.vector.tensor_add(sc_ps[:], sc_ps[:], m_sb[:])
                            exp_sb = exp_pool.tile([128, 512], BF16, tag="exp")
                            nc.scalar.activation(
                                exp_sb[:], sc_ps[:], EXPF, scale=SCALE
                            )
                            nc.tensor.matmul(
                                av_ps[:],
                                v_sb[:, ki * HD : (ki + 1) * HD],
                                exp_sb[:],
                                start=(i == 0),
                                stop=(i == len(kept) - 1),
                            )
                            if i == 0:
                                nc.vector.tensor_copy(acc_sb[:], exp_sb[:])
                            else:
                                nc.vector.tensor_add(acc_sb[:], acc_sb[:], exp_sb[:])
                        nc.tensor.matmul(
                            sm_ps[:], onescr_sb[:], acc_sb[:], start=True, stop=True
                        )
                        # stash unnormalised AV and 1/sums; normalise after
                        # the pair's j-loop so the broadcast matmul never
                        # stalls PE behind the DVE reciprocal
                        with nc.allow_low_precision(
                            reason="f32r == fp32 storage, rounded mantissa"
                        ):
                            nc.vector.reciprocal(
                                rc_all[:, j * 512 : (j + 1) * 512], sm_ps[:]
                            )
                        nc.vector.tensor_copy(
                            aoT[hl][:, b * S + j * 512 : b * S + (j + 1) * 512],
                            av_ps[:],
                        )
                    for j in range(NQ):
                        bc_ps = bc_pool.tile([128, 512], F32, tag="bc")
                        nc.tensor.matmul(
                            bc_ps[:],
                            onesr_sb[:],
                            rc_all[:, j * 512 : (j + 1) * 512],
                            start=True,
                            stop=True,
                        )
                        bc_sb = nrm_pool.tile([128, 512], F32, tag="bcs")
                        nc.vector.tensor_copy(bc_sb[:], bc_ps[:])
                        asl = aoT[hl][:, b * S + j * 512 : b * S + (j + 1) * 512]
                        nc.vector.tensor_mul(asl, asl, bc_sb[:])

        # ═════════ Phase C: partial o_proj  y = aoT.T @ woT ═════════
        with ExitStack() as cctx:
            yo_pool = cctx.enter_context(tc.tile_pool(name="yout", bufs=3))
            yp_pool = cctx.enter_context(
                tc.tile_pool(name="psY", bufs=3, space="PSUM")
            )
            for mt in range(T // 128):
                msl = slice(mt * 128, (mt + 1) * 128)
                for n in range(H // 512):
                    ps = yp_pool.tile([128, 512], F32, tag="y")
                    for hl in range(HPC):
                        nc.tensor.matmul(
                            ps[:],
                            aoT[hl][:, msl],
                            wo_sb[:, hl * H + n * 512 : hl * H + (n + 1) * 512],
                            start=(hl == 0),
                            stop=(hl == HPC - 1),
                        )
                    y_sb = yo_pool.tile([128, 512], F32, tag="ysb")
                    nc.vector.tensor_copy(y_sb[:], ps[:])
                    nc.sync.dma_start(
                        out=y[msl, n * 512 : (n + 1) * 512], in_=y_sb[:]
                    )

    return nc


_CACHE: dict = {}


def _get_nc(mode: str) -> bass.Bass:
    if mode not in _CACHE:
        _CACHE[mode] = _build(mode)
    return _CACHE[mode]


def _rope_tables():
    inv_freq = 1.0 / (THETA ** (np.arange(0, HD, 2, dtype=np.float32) / HD))
    t = np.arange(S, dtype=np.float32)
    freqs = np.einsum("i,j->ij", t, inv_freq)
    emb = np.concatenate((freqs, freqs), axis=-1)  # [S, HD]
    return np.cos(emb), np.sin(emb)


def kernel(hidden_states, attention_mask, Wq, Wk, Wv, Wo):
    hs = np.asarray(hidden_states, dtype=np.float32)
    mask = np.asarray(attention_mask, dtype=np.float32)[0, 0]
    Wq = np.asarray(Wq, dtype=np.float32)
    Wk = np.asarray(Wk, dtype=np.float32)
    Wv = np.asarray(Wv, dtype=np.float32)
    Wo = np.asarray(Wo, dtype=np.float32)

    # ── mask analysis ──
    causal = np.triu(np.full((S, S), -1e9, dtype=np.float32), k=1)
    if np.array_equal(mask, causal):
        mode = "causal"
    elif not mask.any():
        mode = "zeros"
    else:
        mode = "general"

    # ── host-side prep ──
    xt = np.ascontiguousarray(hs.reshape(T, H).T).astype(BF)  # [H, T]
    cos, sin = _rope_tables()  # [S, HD] fp32
    cost = np.ascontiguousarray(np.tile(cos.T, (1, B)))  # [HD, T] fp32
    sint = np.ascontiguousarray(np.tile(sin.T, (1, B)))
    # rotate_half as matmul: rot = P @ qT with P[i, i+64] = -1, P[i+64, i] = 1
    P = np.zeros((HD, HD), dtype=np.float32)
    for i in range(HD // 2):
        P[i, i + HD // 2] = -1.0
        P[i + HD // 2, i] = 1.0
    ptm = np.ascontiguousarray(P.T).astype(BF)
    ones_bf = np.ones((128, 1), dtype=BF)
    ones_f = np.ones((1, 128), dtype=np.float32)
    ones_cf = np.ones((128, 1), dtype=np.float32)

    common = {
        "cost": cost,
        "sint": sint,
        "pt": ptm,
        "ones_bf": ones_bf,
        "ones_f": ones_f,
        "ones_cf": ones_cf.reshape(128, 1),
    }
    if mode == "causal":
        # 4 diagonal tile patterns [128, 512]: pattern r masks where
        # 128*r + p > c  (pre-scaled by sqrt(HD) since exp() applies
        # scale to mask+scores together)
        p_idx = np.arange(128)[:, None]
        c_idx = np.arange(512)[None, :]
        md = np.stack(
            [
                np.where(128 * r + p_idx > c_idx, np.float32(-1e9 * np.sqrt(HD)), 0.0)
                for r in range(4)
            ]
        ).astype(BF)
        common["mdiag"] = np.ascontiguousarray(md.reshape(4 * 128, 512))
    elif mode == "general":
        common["maskt"] = np.ascontiguousarray(mask.T * np.float32(np.sqrt(HD)))

    in_maps = []
    for c in range(NCORES):
        osl = slice(OC * c, OC * (c + 1))
        in_maps.append(
            dict(
                common,
                xt=xt,
                wq=np.ascontiguousarray(Wq[osl, :].T).astype(BF),
                wk=np.ascontiguousarray(Wk[osl, :].T).astype(BF),
                wv=np.ascontiguousarray(Wv[osl, :].T).astype(BF),
                wo=np.ascontiguousarray(Wo[:, osl].T).astype(BF),
            )
        )

    global _last_in_maps
    _last_in_maps = in_maps
    nc = _get_nc(mode)
    res = run_bass_kernel_spmd(nc, in_maps, list(range(NCORES)))
    out = np.zeros((T, H), dtype=np.float32)
    for c in range(NCORES):
        out += res.results[c]["y"]
    return out.reshape(B, S, H)


# revision 11
# speedup vs baseline: 1.0960x; 1.0960x over previous
"""Trainium2 Bass kernel for AliceAttention (dense transformer attention layer).

Reference computation (fp32):
    q/k/v = hidden @ W{q,k,v}.T  -> [B,S,NH,HD], RoPE(q,k),
    scores = q k^T / sqrt(HD) + mask, softmax, out = attn @ v,
    y = out @ Wo.T

Sharding: tensor-parallel over the 32 heads -> 4 heads per core across 8
NeuronCores. Each core computes q/k/v for its heads (columns of the
projections), full attention for its 8 (batch, head) pairs, and a partial
o_proj ( y_c = ao_c @ Wo[:, cols_c].T ); the 8 fp32 partials are summed on
the host.

Device layout choices:
  * All big matmuls run in bf16 (PE streams 1 column/cycle; fp32 is 4x
    slower). PSUM accumulation is fp32.
  * q,k are produced directly in transposed layout qT/kT = [d, t] by using
    W.T slices as the stationary operand. RoPE's rotate_half becomes a
    [128,128] +/-1 permutation matmul (P @ qT) plus elementwise combines.
  * Scores are computed transposed, scores_T = [t_k, t_q] , so that
    (a) attn @ v needs no transposes: outT[d, t_q] = v[t_k, d].T @ exp_T,
    (b) softmax denominators are a ones-column matmul over the partition
        axis, accumulated in PSUM alongside the AV matmul.
    Normalisation is deferred to after AV: outT *= (1/sums) broadcast
    across partitions via a K=1 ones matmul (float32r, exact-ish).
  * Causal masking: strictly-masked [t_k, t_q] tiles are skipped entirely;
    diagonal tiles add one of 4 precomputed [128,512] mask patterns. A
    general (non-causal) additive mask falls back to streaming mask tiles
    for every block; an all-zero mask skips masking but computes all
    blocks.
"""

import sys

import numpy as np
import ml_dtypes
from contextlib import ExitStack

import orjson

import concourse.bass as bass
import concourse.mybir as mybir
import concourse.tile as tile
import concourse.bass2jax as bass2jax
from concourse.bass_utils import run_bass_kernel_spmd

# ─────────────────────────────────────────────────────────────────────────
# This container's walrus rejects instructions carrying more semaphore
# waits than their ISA struct can hold (e.g. the Tile tail-drain with 5).
# Split excess waits into preceding wait-only EventSemaphore instructions
# (2 waits each) on the same engine — semantically identical.
# ─────────────────────────────────────────────────────────────────────────
_WAIT_CAP = {"EventSemaphore": 2}
_DEFAULT_WAIT_CAP = 1


def _legalize_bir_waits(bir_bytes: bytes) -> bytes:
    d = orjson.loads(bir_bytes)
    changed = False
    for fn in d.get("functions", []):
        for blk in fn.get("blocks", []):
            insts = blk.get("instructions")
            if not insts:
                continue
            out = []
            for inst in insts:
                si = inst.get("sync_info")
                waits = (si or {}).get("on_wait") or []
                cap = _WAIT_CAP.get(inst.get("opcode"), _DEFAULT_WAIT_CAP)
                if len(waits) > cap:
                    excess, keep = waits[:-cap], waits[-cap:]
                    for i in range(0, len(excess), 2):
                        out.append(
                            {
                                "debug": inst.get("debug"),
                                "engine": inst["engine"],
                                "ins": [],
                                "outs": [],
                                "name": f"{inst['name']}_xw{i}",
                                "opcode": "EventSemaphore",
                                "sync_info": {
                                    "on_update": [],
                                    "on_wait": excess[i : i + 2],
                                },
                            }
                        )
                    si["on_wait"] = keep
                    changed = True
                out.append(inst)
            blk["instructions"] = out
    return orjson.dumps(d) if changed else bir_bytes


if not getattr(bass2jax, "_wait_legalize_patched", False):
    _orig_compile_bir_kernel = bass2jax.compile_bir_kernel

    def _patched_compile_bir_kernel(ant_bir_str, compile_dir_path, **kw):
        return _orig_compile_bir_kernel(
            _legalize_bir_waits(ant_bir_str), compile_dir_path, **kw
        )

    bass2jax.compile_bir_kernel = _patched_compile_bir_kernel
    bass2jax._wait_legalize_patched = True

# ─────────────────────────────────────────────────────────────────────────
# Problem constants (hardcoded per contract)
# ─────────────────────────────────────────────────────────────────────────
B, S, H, NH, HD = 2, 2048, 4096, 32, 128
THETA = 10000.0
NCORES = 8
HPC = NH // NCORES          # heads per core = 4
OC = HPC * HD               # output cols per core = 512
T = B * S                   # 4096 tokens
KT = H // 128               # 32 contraction tiles for projections
TB = 512                    # t-block width in phase A
NTB = T // TB               # 8 t-blocks
NQ = S // 512               # 4 query blocks per pair
NK = S // 128               # 16 key tiles per pair
SCALE = 1.0 / float(np.sqrt(HD))

F32 = mybir.dt.float32
F32R = mybir.dt.float32r
BF16 = mybir.dt.bfloat16
BF = ml_dtypes.bfloat16
EXPF = mybir.ActivationFunctionType.Exp


def _build(mode: str) -> bass.Bass:
    """mode: 'causal' (skip masked tiles, 4 diag patterns),
    'zeros' (no mask, all tiles), 'general' (stream fp32 mask tiles)."""
    nc = bass.Bass()

    xt = nc.declare_dram_parameter("xt", [H, T], BF16, isOutput=False)
    wq = nc.declare_dram_parameter("wq", [H, OC], BF16, isOutput=False)
    wk = nc.declare_dram_parameter("wk", [H, OC], BF16, isOutput=False)
    wv = nc.declare_dram_parameter("wv", [H, OC], BF16, isOutput=False)
    wo = nc.declare_dram_parameter("wo", [OC, H], BF16, isOutput=False)
    cost = nc.declare_dram_parameter("cost", [HD, T], F32, isOutput=False)
    sint = nc.declare_dram_parameter("sint", [HD, T], F32, isOutput=False)
    pt = nc.declare_dram_parameter("pt", [HD, HD], BF16, isOutput=False)
    ones_bf = nc.declare_dram_parameter("ones_bf", [128, 1], BF16, isOutput=False)
    ones_f = nc.declare_dram_parameter("ones_f", [1, 128], F32, isOutput=False)
    ones_cf = nc.declare_dram_parameter("ones_cf", [128, 1], F32, isOutput=False)
    if mode == "causal":
        mdiag = nc.declare_dram_parameter("mdiag", [4 * 128, 512], BF16, isOutput=False)
    elif mode == "general":
        maskt = nc.declare_dram_parameter("maskt", [S, S], F32, isOutput=False)
    y = nc.declare_dram_parameter("y", [T, H], F32, isOutput=True)

    # DRAM scratch (per core): roped qT/kT [OC, T] (f32r) and v [T, OC] (bf16)
    qts = nc.dram_tensor("qts", [OC, T], BF16)
    kts = nc.dram_tensor("kts", [OC, T], BF16)
    vs = nc.dram_tensor("vs", [T, OC], BF16)

    with tile.TileContext(nc) as tc, ExitStack() as octx:
        # ── pools that live for the whole kernel ──
        const_pool = octx.enter_context(tc.tile_pool(name="const", bufs=1))

        pt_sb = const_pool.tile([HD, HD], BF16)
        nc.sync.dma_start(out=pt_sb[:], in_=pt[:])
        ones_sb = const_pool.tile([128, 1], BF16)
        nc.sync.dma_start(out=ones_sb[:], in_=ones_bf[:])
        onesf_sb = const_pool.tile([1, 128], F32)
        nc.sync.dma_start(out=onesf_sb[:], in_=ones_f[:])
        onesr_sb = const_pool.tile([1, 128], F32R)
        nc.vector.tensor_copy(onesr_sb[:], onesf_sb[:])
        onescf_sb = const_pool.tile([128, 1], F32)
        nc.sync.dma_start(out=onescf_sb[:], in_=ones_cf[:])
        onescr_sb = const_pool.tile([128, 1], F32R)
        nc.vector.tensor_copy(onescr_sb[:], onescf_sb[:])
        if mode == "causal":
            md_sb = const_pool.tile([128, 4 * 512], BF16)
            nc.sync.dma_start(
                out=md_sb[:].rearrange("p (r c) -> p r c", r=4),
                in_=mdiag.rearrange("(r p) c -> p r c", p=128),
            )

        # ═════════ Phase A: QKV projections + RoPE, spill to DRAM ═════════
        with ExitStack() as actx:
            w_pool = actx.enter_context(tc.tile_pool(name="wqkv", bufs=1))
            x_pool = actx.enter_context(tc.tile_pool(name="xblk", bufs=2))
            cs_pool = actx.enter_context(tc.tile_pool(name="cosin", bufs=2))
            ev_pool = actx.enter_context(tc.tile_pool(name="evac", bufs=3))
            rp_pool = actx.enter_context(tc.tile_pool(name="rope", bufs=3))
            ps_pool = actx.enter_context(
                tc.tile_pool(name="psA", bufs=3, space="PSUM")
            )
            rot_pool = actx.enter_context(
                tc.tile_pool(name="psRot", bufs=2, space="PSUM")
            )

            # weights resident: [128, k*OC + o] layouts; wq first so the
            # first accumulation can start as early as possible
            wq_sb = w_pool.tile([128, KT * OC], BF16, tag="wq")
            wk_sb = w_pool.tile([128, KT * OC], BF16, tag="wk")
            wv_sb = w_pool.tile([128, KT * OC], BF16, tag="wv")
            nc.sync.dma_start(
                out=wq_sb[:].rearrange("p (k o) -> p k o", k=KT),
                in_=wq.rearrange("(k p) o -> p k o", p=128),
            )

            for tb in range(NTB):
                tsl = slice(tb * TB, (tb + 1) * TB)
                x_sb = x_pool.tile([128, KT * TB], BF16, tag="x")
                nc.sync.dma_start(
                    out=x_sb[:].rearrange("p (k t) -> p k t", k=KT),
                    in_=xt[:, tsl].rearrange("(k p) t -> p k t", p=128),
                )
                cos_sb = cs_pool.tile([HD, TB], F32, tag="cos")
                sin_sb = cs_pool.tile([HD, TB], F32, tag="sin")
                nc.sync.dma_start(out=cos_sb[:], in_=cost[:, tsl])
                nc.sync.dma_start(out=sin_sb[:], in_=sint[:, tsl])
                if tb == 0:
                    for w_dram, w_sb in ((wk, wk_sb), (wv, wv_sb)):
                        nc.sync.dma_start(
                            out=w_sb[:].rearrange("p (k o) -> p k o", k=KT),
                            in_=w_dram.rearrange("(k p) o -> p k o", p=128),
                        )

                # q and k: out tiles [o 128, t 512], RoPE, spill
                for which, w_sb, spill in (("q", wq_sb, qts), ("k", wk_sb, kts)):
                    for ot in range(HPC):
                        ps = ps_pool.tile([128, TB], F32, tag="proj")
                        for k in range(KT):
                            nc.tensor.matmul(
                                ps[:],
                                w_sb[:, k * OC + ot * 128 : k * OC + (ot + 1) * 128],
                                x_sb[:, k * TB : (k + 1) * TB],
                                start=(k == 0),
                                stop=(k == KT - 1),
                            )
                        raw_sb = ev_pool.tile([128, TB], BF16, tag="rawqk")
                        nc.vector.tensor_copy(raw_sb[:], ps[:])
                        rot_ps = rot_pool.tile([128, TB], F32, tag="rot")
                        nc.tensor.matmul(
                            rot_ps[:], pt_sb[:], raw_sb[:], start=True, stop=True
                        )
                        t1 = rp_pool.tile([128, TB], F32, tag="t1")
                        nc.vector.tensor_mul(t1[:], raw_sb[:], cos_sb[:])
                        t2 = rp_pool.tile([128, TB], F32, tag="t2")
                        nc.vector.tensor_mul(t2[:], rot_ps[:], sin_sb[:])
                        roped = ev_pool.tile([128, TB], BF16, tag="roped")
                        nc.vector.tensor_add(roped[:], t1[:], t2[:])
                        nc.sync.dma_start(
                            out=spill[ot * 128 : (ot + 1) * 128, tsl], in_=roped[:]
                        )

                # v: out tiles [t 128, o 512] (natural layout), spill
                for mt in range(TB // 128):
                    ps = ps_pool.tile([128, OC], F32, tag="proj")
                    for k in range(KT):
                        nc.tensor.matmul(
                            ps[:],
                            x_sb[:, k * TB + mt * 128 : k * TB + (mt + 1) * 128],
                            wv_sb[:, k * OC : (k + 1) * OC],
                            start=(k == 0),
                            stop=(k == KT - 1),
                        )
                    v_sb = ev_pool.tile([128, OC], BF16, tag="vout")
                    nc.vector.tensor_copy(v_sb[:], ps[:])
                    nc.sync.dma_start(
                        out=vs[tb * TB + mt * 128 : tb * TB + (mt + 1) * 128, :],
                        in_=v_sb[:],
                    )

        # ── residents for phases B+C (allocated after phase A frees SBUF) ──
        ao_pool = octx.enter_context(tc.tile_pool(name="ao", bufs=1))
        wo_pool = octx.enter_context(tc.tile_pool(name="wo", bufs=1))
        # attention output, transposed: one [128, T] tile per local head
        aoT = [
            ao_pool.tile([HD, T], BF16, tag=f"aoT{hl}", name=f"aoT{hl}")
            for hl in range(HPC)
        ]
        # Wo.T resident: [128, hl*H + hout]
        wo_sb = wo_pool.tile([128, HPC * H], BF16)
        nc.sync.dma_start(
            out=wo_sb[:].rearrange("p (hl n) -> p hl n", hl=HPC),
            in_=wo.rearrange("(hl p) n -> p hl n", p=128),
        )

        # ═════════ Phase B: attention per (batch, local head) ═════════
        with ExitStack() as bctx:
            qk_pool = bctx.enter_context(tc.tile_pool(name="qkv_pair", bufs=2))
            exp_pool = bctx.enter_context(tc.tile_pool(name="exp", bufs=4))
            nrm_pool = bctx.enter_context(tc.tile_pool(name="nrm", bufs=3))
            if mode == "general":
                mt_pool = bctx.enter_context(tc.tile_pool(name="mtile", bufs=4))
            sc_pool = bctx.enter_context(
                tc.tile_pool(name="psSc", bufs=3, space="PSUM")
            )
            av_pool = bctx.enter_context(
                tc.tile_pool(name="psAv", bufs=2, space="PSUM")
            )
            sm_pool = bctx.enter_context(
                tc.tile_pool(name="psSum", bufs=2, space="PSUM")
            )
            bc_pool = bctx.enter_context(
                tc.tile_pool(name="psBc", bufs=1, space="PSUM")
            )

            for b in range(B):
                for hl in range(HPC):
                    qT_sb = qk_pool.tile([HD, S], BF16, tag="qTh")
                    kT_sb = qk_pool.tile([HD, S], BF16, tag="kTh")
                    v_sb = qk_pool.tile([128, NK * HD], BF16, tag="vh")
                    osl = slice(hl * 128, (hl + 1) * 128)
                    bsl = slice(b * S, (b + 1) * S)
                    nc.sync.dma_start(out=qT_sb[:], in_=qts[osl, bsl])
                    nc.sync.dma_start(out=kT_sb[:], in_=kts[osl, bsl])
                    nc.sync.dma_start(
                        out=v_sb[:].rearrange("p (k d) -> p k d", k=NK),
                        in_=vs[bsl, osl].rearrange("(k p) d -> p k d", p=128),
                    )
                    rc_all = nrm_pool.tile([1, NQ * 512], F32R, tag="rcall")

                    for j in range(NQ):
                        if mode == "causal":
                            kept = list(range(min(NK, 4 * j + 4)))
                        else:
                            kept = list(range(NK))
                        qsl = slice(j * 512, (j + 1) * 512)
                        av_ps = av_pool.tile([128, 512], F32, tag="av")
                        sm_ps = sm_pool.tile([1, 512], F32, tag="sm")
                        for i, ki in enumerate(kept):
                            sc_ps = sc_pool.tile([128, 512], F32, tag="sc")
                            nc.tensor.matmul(
                                sc_ps[:],
                                kT_sb[:, ki * 128 : (ki + 1) * 128],
                                qT_sb[:, qsl],
                                start=True,
                                stop=True,
                            )
                            if mode == "causal" and ki >= 4 * j:
                                r = ki - 4 * j
                                nc.vector.tensor_add(
                                    sc_ps[:],
                                    sc_ps[:],
                                    md_sb[:, r * 512 : (r + 1) * 512],
                                )
                            elif mode == "general":
                                m_sb = mt_pool.tile([128, 512], F32, tag="mt")
                                nc.sync.dma_start(
                                    out=m_sb[:],
                                    in_=maskt[ki * 128 : (ki + 1) * 128, qsl],
                                )
                                nc.vector.tensor_add(sc_ps[:], sc_ps[:], m_sb[:])
                            exp_sb = exp_pool.tile([128, 512], BF16, tag="exp")
                            nc.scalar.activation(
                                exp_sb[:], sc_ps[:], EXPF, scale=SCALE
                            )
                            nc.tensor.matmul(
                                av_ps[:],
                                v_sb[:, ki * HD : (ki + 1) * HD],
                                exp_sb[:],
                                start=(i == 0),
                                stop=(i == len(kept) - 1),
                            )
                            nc.tensor.matmul(
                                sm_ps[:],
                                ones_sb[:],
                                exp_sb[:],
                                start=(i == 0),
                                stop=(i == len(kept) - 1),
                            )
                        # stash unnormalised AV and 1/sums; normalise after
                        # the pair's j-loop so the broadcast matmul never
                        # stalls PE behind the DVE reciprocal
                        with nc.allow_low_precision(
                            reason="f32r == fp32 storage, rounded mantissa"
                        ):
                            nc.vector.reciprocal(
                                rc_all[:, j * 512 : (j + 1) * 512], sm_ps[:]
                            )
                        nc.vector.tensor_copy(
                            aoT[hl][:, b * S + j * 512 : b * S + (j + 1) * 512],
                            av_ps[:],
                        )
                    for j in range(NQ):
                        bc_ps = bc_pool.tile([128, 512], F32, tag="bc")
                        nc.tensor.matmul(
                            bc_ps[:],
                            onesr_sb[:],
                            rc_all[:, j * 512 : (j + 1) * 512],
                            start=True,
                            stop=True,
                        )
                        bc_sb = nrm_pool.tile([128, 512], F32, tag="bcs")
                        nc.vector.tensor_copy(bc_sb[:], bc_ps[:])
                        asl = aoT[hl][:, b * S + j * 512 : b * S + (j + 1) * 512]
                        nc.vector.tensor_mul(asl, asl, bc_sb[:])

        # ═════════ Phase C: partial o_proj  y = aoT.T @ woT ═════════
        with ExitStack() as cctx:
            yo_pool = cctx.enter_context(tc.tile_pool(name="yout", bufs=3))
            yp_pool = cctx.enter_context(
                tc.tile_pool(name="psY", bufs=3, space="PSUM")
            )
            for mt in range(T // 128):
                msl = slice(mt * 128, (mt + 1) * 128)
                for n in range(H // 512):
                    ps = yp_pool.tile([128, 512], F32, tag="y")
                    for hl in range(HPC):
                        nc.tensor.matmul(
                            ps[:],
                            aoT[hl][:, msl],
                            wo_sb[:, hl * H + n * 512 : hl * H + (n + 1) * 512],
                            start=(hl == 0),
                            stop=(hl == HPC - 1),
                        )
                    y_sb = yo_pool.tile([128, 512], F32, tag="ysb")
                    nc.vector.tensor_copy(y_sb[:], ps[:])
                    nc.sync.dma_start(
                        out=y[msl, n * 512 : (n + 1) * 512], in_=y_sb[:]
                    )

    return nc


_CACHE: dict = {}


def _get_nc(mode: str) -> bass.Bass:
    if mode not in _CACHE:
        _CACHE[mode] = _build(mode)
    return _CACHE[mode]


def _rope_tables():
    inv_freq = 1.0 / (THETA ** (np.arange(0, HD, 2, dtype=np.float32) / HD))
    t = np.arange(S, dtype=np.float32)
    freqs = np.einsum("i,j->ij", t, inv_freq)
    emb = np.concatenate((freqs, freqs), axis=-1)  # [S, HD]
    return np.cos(emb), np.sin(emb)


def kernel(hidden_states, attention_mask, Wq, Wk, Wv, Wo):
    hs = np.asarray(hidden_states, dtype=np.float32)
    mask = np.asarray(attention_mask, dtype=np.float32)[0, 0]
    Wq = np.asarray(Wq, dtype=np.float32)
    Wk = np.asarray(Wk, dtype=np.float32)
    Wv = np.asarray(Wv, dtype=np.float32)
    Wo = np.asarray(Wo, dtype=np.float32)

    # ── mask analysis ──
    causal = np.triu(np.full((S, S), -1e9, dtype=np.float32), k=1)
    if np.array_equal(mask, causal):
        mode = "causal"
    elif not mask.any():
        mode = "zeros"
    else:
        mode = "general"

    # ── host-side prep ──
    xt = np.ascontiguousarray(hs.reshape(T, H).T).astype(BF)  # [H, T]
    cos, sin = _rope_tables()  # [S, HD] fp32
    cost = np.ascontiguousarray(np.tile(cos.T, (1, B)))  # [HD, T] fp32
    sint = np.ascontiguousarray(np.tile(sin.T, (1, B)))
    # rotate_half as matmul: rot = P @ qT with P[i, i+64] = -1, P[i+64, i] = 1
    P = np.zeros((HD, HD), dtype=np.float32)
    for i in range(HD // 2):
        P[i, i + HD // 2] = -1.0
        P[i + HD // 2, i] = 1.0
    ptm = np.ascontiguousarray(P.T).astype(BF)
    ones_bf = np.ones((128, 1), dtype=BF)
    ones_f = np.ones((1, 128), dtype=np.float32)
    ones_cf = np.ones((128, 1), dtype=np.float32)

    common = {
        "cost": cost,
        "sint": sint,
        "pt": ptm,
        "ones_bf": ones_bf,
        "ones_f": ones_f,
        "ones_cf": ones_cf.reshape(128, 1),
    }
    if mode == "causal":
        # 4 diagonal tile patterns [128, 512]: pattern r masks where
        # 128*r + p > c  (pre-scaled by sqrt(HD) since exp() applies
        # scale to mask+scores together)
        p_idx = np.arange(128)[:, None]
        c_idx = np.arange(512)[None, :]
        md = np.stack(
            [
                np.where(128 * r + p_idx > c_idx, np.float32(-1e9 * np.sqrt(HD)), 0.0)
                for r in range(4)
            ]
        ).astype(BF)
        common["mdiag"] = np.ascontiguousarray(md.reshape(4 * 128, 512))
    elif mode == "general":
        common["maskt"] = np.ascontiguousarray(mask.T * np.float32(np.sqrt(HD)))

    in_maps = []
    for c in range(NCORES):
        osl = slice(OC * c, OC * (c + 1))
        in_maps.append(
            dict(
                common,
                xt=xt,
                wq=np.ascontiguousarray(Wq[osl, :].T).astype(BF),
                wk=np.ascontiguousarray(Wk[osl, :].T).astype(BF),
                wv=np.ascontiguousarray(Wv[osl, :].T).astype(BF),
                wo=np.ascontiguousarray(Wo[:, osl].T).astype(BF),
            )
        )

    global _last_in_maps
    _last_in_maps = in_maps
    nc = _get_nc(mode)
    res = run_bass_kernel_spmd(nc, in_maps, list(range(NCORES)))
    out = np.zeros((T, H), dtype=np.float32)
    for c in range(NCORES):
        out += res.results[c]["y"]
    return out.reshape(B, S, H)


# revision 12
# speedup vs baseline: 1.1190x; 1.0210x over previous
"""Trainium2 Bass kernel for AliceAttention (dense transformer attention layer).

Reference computation (fp32):
    q/k/v = hidden @ W{q,k,v}.T  -> [B,S,NH,HD], RoPE(q,k),
    scores = q k^T / sqrt(HD) + mask, softmax, out = attn @ v,
    y = out @ Wo.T

Sharding: tensor-parallel over the 32 heads -> 4 heads per core across 8
NeuronCores. Each core computes q/k/v for its heads (columns of the
projections), full attention for its 8 (batch, head) pairs, and a partial
o_proj ( y_c = ao_c @ Wo[:, cols_c].T ); the 8 fp32 partials are summed on
the host.

Device layout choices:
  * All big matmuls run in bf16 (PE streams 1 column/cycle; fp32 is 4x
    slower). PSUM accumulation is fp32.
  * q,k are produced directly in transposed layout qT/kT = [d, t] by using
    W.T slices as the stationary operand. RoPE's rotate_half becomes a
    [128,128] +/-1 permutation matmul (P @ qT) plus elementwise combines.
  * Scores are computed transposed, scores_T = [t_k, t_q] , so that
    (a) attn @ v needs no transposes: outT[d, t_q] = v[t_k, d].T @ exp_T,
    (b) softmax denominators are a ones-column matmul over the partition
        axis, accumulated in PSUM alongside the AV matmul.
    Normalisation is deferred to after AV: outT *= (1/sums) broadcast
    across partitions via a K=1 ones matmul (float32r, exact-ish).
  * Causal masking: strictly-masked [t_k, t_q] tiles are skipped entirely;
    diagonal tiles add one of 4 precomputed [128,512] mask patterns. A
    general (non-causal) additive mask falls back to streaming mask tiles
    for every block; an all-zero mask skips masking but computes all
    blocks.
"""

import sys

import numpy as np
import ml_dtypes
from contextlib import ExitStack

import orjson

import concourse.bass as bass
import concourse.mybir as mybir
import concourse.tile as tile
import concourse.bass2jax as bass2jax
from concourse.bass_utils import run_bass_kernel_spmd

# ─────────────────────────────────────────────────────────────────────────
# This container's walrus rejects instructions carrying more semaphore
# waits than their ISA struct can hold (e.g. the Tile tail-drain with 5).
# Split excess waits into preceding wait-only EventSemaphore instructions
# (2 waits each) on the same engine — semantically identical.
# ─────────────────────────────────────────────────────────────────────────
_WAIT_CAP = {"EventSemaphore": 2}
_DEFAULT_WAIT_CAP = 1


def _legalize_bir_waits(bir_bytes: bytes) -> bytes:
    d = orjson.loads(bir_bytes)
    changed = False
    for fn in d.get("functions", []):
        for blk in fn.get("blocks", []):
            insts = blk.get("instructions")
            if not insts:
                continue
            out = []
            for inst in insts:
                si = inst.get("sync_info")
                waits = (si or {}).get("on_wait") or []
                cap = _WAIT_CAP.get(inst.get("opcode"), _DEFAULT_WAIT_CAP)
                if len(waits) > cap:
                    excess, keep = waits[:-cap], waits[-cap:]
                    for i in range(0, len(excess), 2):
                        out.append(
                            {
                                "debug": inst.get("debug"),
                                "engine": inst["engine"],
                                "ins": [],
                                "outs": [],
                                "name": f"{inst['name']}_xw{i}",
                                "opcode": "EventSemaphore",
                                "sync_info": {
                                    "on_update": [],
                                    "on_wait": excess[i : i + 2],
                                },
                            }
                        )
                    si["on_wait"] = keep
                    changed = True
                out.append(inst)
            blk["instructions"] = out
    return orjson.dumps(d) if changed else bir_bytes


if not getattr(bass2jax, "_wait_legalize_patched", False):
    _orig_compile_bir_kernel = bass2jax.compile_bir_kernel

    def _patched_compile_bir_kernel(ant_bir_str, compile_dir_path, **kw):
        return _orig_compile_bir_kernel(
            _legalize_bir_waits(ant_bir_str), compile_dir_path, **kw
        )

    bass2jax.compile_bir_kernel = _patched_compile_bir_kernel
    bass2jax._wait_legalize_patched = True

# ─────────────────────────────────────────────────────────────────────────
# Problem constants (hardcoded per contract)
# ─────────────────────────────────────────────────────────────────────────
B, S, H, NH, HD = 2, 2048, 4096, 32, 128
THETA = 10000.0
NCORES = 8
HPC = NH // NCORES          # heads per core = 4
OC = HPC * HD               # output cols per core = 512
T = B * S                   # 4096 tokens
KT = H // 128               # 32 contraction tiles for projections
TB = 512                    # t-block width in phase A
NTB = T // TB               # 8 t-blocks
NQ = S // 512               # 4 query blocks per pair
NK = S // 128               # 16 key tiles per pair
SCALE = 1.0 / float(np.sqrt(HD))

F32 = mybir.dt.float32
F32R = mybir.dt.float32r
BF16 = mybir.dt.bfloat16
BF = ml_dtypes.bfloat16
EXPF = mybir.ActivationFunctionType.Exp


def _build(mode: str) -> bass.Bass:
    """mode: 'causal' (skip masked tiles, 4 diag patterns),
    'zeros' (no mask, all tiles), 'general' (stream fp32 mask tiles)."""
    nc = bass.Bass()

    xt = nc.declare_dram_parameter("xt", [H, T], BF16, isOutput=False)
    wq = nc.declare_dram_parameter("wq", [H, OC], BF16, isOutput=False)
    wk = nc.declare_dram_parameter("wk", [H, OC], BF16, isOutput=False)
    wv = nc.declare_dram_parameter("wv", [H, OC], BF16, isOutput=False)
    wo = nc.declare_dram_parameter("wo", [OC, H], BF16, isOutput=False)
    cost = nc.declare_dram_parameter("cost", [HD, T], BF16, isOutput=False)
    sint = nc.declare_dram_parameter("sint", [HD, T], BF16, isOutput=False)
    pt = nc.declare_dram_parameter("pt", [HD, HD], BF16, isOutput=False)
    ones_bf = nc.declare_dram_parameter("ones_bf", [128, 128], BF16, isOutput=False)
    if mode == "causal":
        mdiag = nc.declare_dram_parameter("mdiag", [4 * 128, 512], BF16, isOutput=False)
    elif mode == "general":
        maskt = nc.declare_dram_parameter("maskt", [S, S], F32, isOutput=False)
    y = nc.declare_dram_parameter("y", [T, H], F32, isOutput=True)

    # DRAM scratch (per core): roped qT/kT [OC, T] (f32r) and v [T, OC] (bf16)
    qts = nc.dram_tensor("qts", [OC, T], BF16)
    kts = nc.dram_tensor("kts", [OC, T], BF16)
    vs = nc.dram_tensor("vs", [T, OC], BF16)

    with tile.TileContext(nc) as tc, ExitStack() as octx:
        # ── pools that live for the whole kernel ──
        const_pool = octx.enter_context(tc.tile_pool(name="const", bufs=1))

        pt_sb = const_pool.tile([HD, HD], BF16)
        nc.sync.dma_start(out=pt_sb[:], in_=pt[:])
        ones_sb = const_pool.tile([128, 128], BF16)
        nc.sync.dma_start(out=ones_sb[:], in_=ones_bf[:])
        if mode == "causal":
            md_sb = const_pool.tile([128, 4 * 512], BF16)
            nc.sync.dma_start(
                out=md_sb[:].rearrange("p (r c) -> p r c", r=4),
                in_=mdiag.rearrange("(r p) c -> p r c", p=128),
            )

        # ═════════ Phase A: QKV projections + RoPE, spill to DRAM ═════════
        with ExitStack() as actx:
            w_pool = actx.enter_context(tc.tile_pool(name="wqkv", bufs=1))
            x_pool = actx.enter_context(tc.tile_pool(name="xblk", bufs=2))
            cs_pool = actx.enter_context(tc.tile_pool(name="cosin", bufs=2))
            ev_pool = actx.enter_context(tc.tile_pool(name="evac", bufs=3))
            rp_pool = actx.enter_context(tc.tile_pool(name="rope", bufs=3))
            ps_pool = actx.enter_context(
                tc.tile_pool(name="psA", bufs=3, space="PSUM")
            )
            rot_pool = actx.enter_context(
                tc.tile_pool(name="psRot", bufs=2, space="PSUM")
            )

            # weights resident: [128, k*OC + o] layouts; wq first so the
            # first accumulation can start as early as possible
            wq_sb = w_pool.tile([128, KT * OC], BF16, tag="wq")
            wk_sb = w_pool.tile([128, KT * OC], BF16, tag="wk")
            wv_sb = w_pool.tile([128, KT * OC], BF16, tag="wv")
            nc.sync.dma_start(
                out=wq_sb[:].rearrange("p (k o) -> p k o", k=KT),
                in_=wq.rearrange("(k p) o -> p k o", p=128),
            )

            for tb in range(NTB):
                tsl = slice(tb * TB, (tb + 1) * TB)
                x_sb = x_pool.tile([128, KT * TB], BF16, tag="x")
                nc.sync.dma_start(
                    out=x_sb[:].rearrange("p (k t) -> p k t", k=KT),
                    in_=xt[:, tsl].rearrange("(k p) t -> p k t", p=128),
                )
                cos_sb = cs_pool.tile([HD, TB], BF16, tag="cos")
                sin_sb = cs_pool.tile([HD, TB], BF16, tag="sin")
                nc.sync.dma_start(out=cos_sb[:], in_=cost[:, tsl])
                nc.sync.dma_start(out=sin_sb[:], in_=sint[:, tsl])
                if tb == 0:
                    for w_dram, w_sb in ((wk, wk_sb), (wv, wv_sb)):
                        nc.sync.dma_start(
                            out=w_sb[:].rearrange("p (k o) -> p k o", k=KT),
                            in_=w_dram.rearrange("(k p) o -> p k o", p=128),
                        )

                # q and k: accumulate all 8 o-tiles first (dense PE), then
                # rot-matmuls read long-finished DVE copies - no PE bubbles
                raws = []
                for which, w_sb, spill in (("q", wq_sb, qts), ("k", wk_sb, kts)):
                    for ot in range(HPC):
                        ps = ps_pool.tile([128, TB], F32, tag="proj")
                        for k in range(KT):
                            nc.tensor.matmul(
                                ps[:],
                                w_sb[:, k * OC + ot * 128 : k * OC + (ot + 1) * 128],
                                x_sb[:, k * TB : (k + 1) * TB],
                                start=(k == 0),
                                stop=(k == KT - 1),
                            )
                        raw_sb = ev_pool.tile(
                            [128, TB], BF16, tag="rawqk", name=f"raw{which}{ot}"
                        )
                        nc.vector.tensor_copy(raw_sb[:], ps[:])
                        raws.append((raw_sb, spill, ot))
                for raw_sb, spill, ot in raws:
                    rot_ps = rot_pool.tile([128, TB], F32, tag="rot")
                    nc.tensor.matmul(
                        rot_ps[:], pt_sb[:], raw_sb[:], start=True, stop=True
                    )
                    t1 = rp_pool.tile([128, TB], F32, tag="t1")
                    nc.vector.tensor_mul(t1[:], raw_sb[:], cos_sb[:])
                    t2 = rp_pool.tile([128, TB], F32, tag="t2")
                    nc.vector.tensor_mul(t2[:], rot_ps[:], sin_sb[:])
                    roped = ev_pool.tile([128, TB], BF16, tag="roped")
                    nc.vector.tensor_add(roped[:], t1[:], t2[:])
                    nc.sync.dma_start(
                        out=spill[ot * 128 : (ot + 1) * 128, tsl], in_=roped[:]
                    )

                # v: out tiles [t 128, o 512] (natural layout), spill
                for mt in range(TB // 128):
                    ps = ps_pool.tile([128, OC], F32, tag="proj")
                    for k in range(KT):
                        nc.tensor.matmul(
                            ps[:],
                            x_sb[:, k * TB + mt * 128 : k * TB + (mt + 1) * 128],
                            wv_sb[:, k * OC : (k + 1) * OC],
                            start=(k == 0),
                            stop=(k == KT - 1),
                        )
                    v_sb = ev_pool.tile([128, OC], BF16, tag="vout")
                    nc.vector.tensor_copy(v_sb[:], ps[:])
                    nc.sync.dma_start(
                        out=vs[tb * TB + mt * 128 : tb * TB + (mt + 1) * 128, :],
                        in_=v_sb[:],
                    )

        # ── residents for phases B+C (allocated after phase A frees SBUF) ──
        ao_pool = octx.enter_context(tc.tile_pool(name="ao", bufs=1))
        wo_pool = octx.enter_context(tc.tile_pool(name="wo", bufs=1))
        # attention output, transposed: one [128, T] tile per local head
        aoT = [
            ao_pool.tile([HD, T], BF16, tag=f"aoT{hl}", name=f"aoT{hl}")
            for hl in range(HPC)
        ]
        # Wo.T resident: [128, hl*H + hout]
        wo_sb = wo_pool.tile([128, HPC * H], BF16)
        nc.sync.dma_start(
            out=wo_sb[:].rearrange("p (hl n) -> p hl n", hl=HPC),
            in_=wo.rearrange("(hl p) n -> p hl n", p=128),
        )

        # ═════════ Phase B: attention per (batch, local head) ═════════
        with ExitStack() as bctx:
            qk_pool = bctx.enter_context(tc.tile_pool(name="qkv_pair", bufs=2))
            exp_pool = bctx.enter_context(tc.tile_pool(name="exp", bufs=4))
            nrm_pool = bctx.enter_context(tc.tile_pool(name="nrm", bufs=3))
            if mode == "general":
                mt_pool = bctx.enter_context(tc.tile_pool(name="mtile", bufs=4))
            sc_pool = bctx.enter_context(
                tc.tile_pool(name="psSc", bufs=3, space="PSUM")
            )
            av_pool = bctx.enter_context(
                tc.tile_pool(name="psAv", bufs=2, space="PSUM")
            )
            sm_pool = bctx.enter_context(
                tc.tile_pool(name="psSum", bufs=2, space="PSUM")
            )

            for b in range(B):
                for hl in range(HPC):
                    qT_sb = qk_pool.tile([HD, S], BF16, tag="qTh")
                    kT_sb = qk_pool.tile([HD, S], BF16, tag="kTh")
                    v_sb = qk_pool.tile([128, NK * HD], BF16, tag="vh")
                    osl = slice(hl * 128, (hl + 1) * 128)
                    bsl = slice(b * S, (b + 1) * S)
                    nc.sync.dma_start(out=qT_sb[:], in_=qts[osl, bsl])
                    nc.sync.dma_start(out=kT_sb[:], in_=kts[osl, bsl])
                    nc.sync.dma_start(
                        out=v_sb[:].rearrange("p (k d) -> p k d", k=NK),
                        in_=vs[bsl, osl].rearrange("(k p) d -> p k d", p=128),
                    )

                    for j in range(NQ):
                        if mode == "causal":
                            kept = list(range(min(NK, 4 * j + 4)))
                        else:
                            kept = list(range(NK))
                        qsl = slice(j * 512, (j + 1) * 512)
                        av_ps = av_pool.tile([128, 512], F32, tag="av")
                        sm_ps = sm_pool.tile([128, 512], F32, tag="sm")
                        for i, ki in enumerate(kept):
                            sc_ps = sc_pool.tile([128, 512], F32, tag="sc")
                            nc.tensor.matmul(
                                sc_ps[:],
                                kT_sb[:, ki * 128 : (ki + 1) * 128],
                                qT_sb[:, qsl],
                                start=True,
                                stop=True,
                            )
                            if mode == "causal" and ki >= 4 * j:
                                r = ki - 4 * j
                                nc.vector.tensor_add(
                                    sc_ps[:],
                                    sc_ps[:],
                                    md_sb[:, r * 512 : (r + 1) * 512],
                                )
                            elif mode == "general":
                                m_sb = mt_pool.tile([128, 512], F32, tag="mt")
                                nc.sync.dma_start(
                                    out=m_sb[:],
                                    in_=maskt[ki * 128 : (ki + 1) * 128, qsl],
                                )
                                nc.vector.tensor_add(sc_ps[:], sc_ps[:], m_sb[:])
                            exp_sb = exp_pool.tile([128, 512], BF16, tag="exp")
                            nc.scalar.activation(
                                exp_sb[:], sc_ps[:], EXPF, scale=SCALE
                            )
                            nc.tensor.matmul(
                                av_ps[:],
                                v_sb[:, ki * HD : (ki + 1) * HD],
                                exp_sb[:],
                                start=(i == 0),
                                stop=(i == len(kept) - 1),
                            )
                            nc.tensor.matmul(
                                sm_ps[:],
                                ones_sb[:],
                                exp_sb[:],
                                start=(i == 0),
                                stop=(i == len(kept) - 1),
                            )
                        # sums arrive pre-broadcast across partitions (ones
                        # lhsT is [128,128]); reciprocal + multiply on DVE
                        rc_sb = nrm_pool.tile([128, 512], F32, tag="rc")
                        nc.vector.reciprocal(rc_sb[:], sm_ps[:])
                        nc.vector.tensor_mul(
                            aoT[hl][:, b * S + j * 512 : b * S + (j + 1) * 512],
                            av_ps[:],
                            rc_sb[:],
                        )

        # ═════════ Phase C: partial o_proj  y = aoT.T @ woT ═════════
        with ExitStack() as cctx:
            yo_pool = cctx.enter_context(tc.tile_pool(name="yout", bufs=3))
            yp_pool = cctx.enter_context(
                tc.tile_pool(name="psY", bufs=3, space="PSUM")
            )
            for mt in range(T // 128):
                msl = slice(mt * 128, (mt + 1) * 128)
                for n in range(H // 512):
                    ps = yp_pool.tile([128, 512], F32, tag="y")
                    for hl in range(HPC):
                        nc.tensor.matmul(
                            ps[:],
                            aoT[hl][:, msl],
                            wo_sb[:, hl * H + n * 512 : hl * H + (n + 1) * 512],
                            start=(hl == 0),
                            stop=(hl == HPC - 1),
                        )
                    y_sb = yo_pool.tile([128, 512], F32, tag="ysb")
                    nc.vector.tensor_copy(y_sb[:], ps[:])
                    nc.sync.dma_start(
                        out=y[msl, n * 512 : (n + 1) * 512], in_=y_sb[:]
                    )

    return nc


_CACHE: dict = {}


def _get_nc(mode: str) -> bass.Bass:
    if mode not in _CACHE:
        _CACHE[mode] = _build(mode)
    return _CACHE[mode]


def _rope_tables():
    inv_freq = 1.0 / (THETA ** (np.arange(0, HD, 2, dtype=np.float32) / HD))
    t = np.arange(S, dtype=np.float32)
    freqs = np.einsum("i,j->ij", t, inv_freq)
    emb = np.concatenate((freqs, freqs), axis=-1)  # [S, HD]
    return np.cos(emb), np.sin(emb)


def kernel(hidden_states, attention_mask, Wq, Wk, Wv, Wo):
    hs = np.asarray(hidden_states, dtype=np.float32)
    mask = np.asarray(attention_mask, dtype=np.float32)[0, 0]
    Wq = np.asarray(Wq, dtype=np.float32)
    Wk = np.asarray(Wk, dtype=np.float32)
    Wv = np.asarray(Wv, dtype=np.float32)
    Wo = np.asarray(Wo, dtype=np.float32)

    # ── mask analysis ──
    causal = np.triu(np.full((S, S), -1e9, dtype=np.float32), k=1)
    if np.array_equal(mask, causal):
        mode = "causal"
    elif not mask.any():
        mode = "zeros"
    else:
        mode = "general"

    # ── host-side prep ──
    xt = np.ascontiguousarray(hs.reshape(T, H).T).astype(BF)  # [H, T]
    cos, sin = _rope_tables()  # [S, HD] fp32
    cost = np.ascontiguousarray(np.tile(cos.T, (1, B))).astype(BF)  # [HD, T]
    sint = np.ascontiguousarray(np.tile(sin.T, (1, B))).astype(BF)
    # rotate_half as matmul: rot = P @ qT with P[i, i+64] = -1, P[i+64, i] = 1
    P = np.zeros((HD, HD), dtype=np.float32)
    for i in range(HD // 2):
        P[i, i + HD // 2] = -1.0
        P[i + HD // 2, i] = 1.0
    ptm = np.ascontiguousarray(P.T).astype(BF)
    ones_bf = np.ones((128, 128), dtype=BF)

    common = {
        "cost": cost,
        "sint": sint,
        "pt": ptm,
        "ones_bf": ones_bf,
    }
    if mode == "causal":
        # 4 diagonal tile patterns [128, 512]: pattern r masks where
        # 128*r + p > c  (pre-scaled by sqrt(HD) since exp() applies
        # scale to mask+scores together)
        p_idx = np.arange(128)[:, None]
        c_idx = np.arange(512)[None, :]
        md = np.stack(
            [
                np.where(128 * r + p_idx > c_idx, np.float32(-1e9 * np.sqrt(HD)), 0.0)
                for r in range(4)
            ]
        ).astype(BF)
        common["mdiag"] = np.ascontiguousarray(md.reshape(4 * 128, 512))
    elif mode == "general":
        common["maskt"] = np.ascontiguousarray(mask.T * np.float32(np.sqrt(HD)))

    in_maps = []
    for c in range(NCORES):
        osl = slice(OC * c, OC * (c + 1))
        in_maps.append(
            dict(
                common,
                xt=xt,
                wq=np.ascontiguousarray(Wq[osl, :].T).astype(BF),
                wk=np.ascontiguousarray(Wk[osl, :].T).astype(BF),
                wv=np.ascontiguousarray(Wv[osl, :].T).astype(BF),
                wo=np.ascontiguousarray(Wo[:, osl].T).astype(BF),
            )
        )

    global _last_in_maps
    _last_in_maps = in_maps
    nc = _get_nc(mode)
    res = run_bass_kernel_spmd(nc, in_maps, list(range(NCORES)))
    out = np.zeros((T, H), dtype=np.float32)
    for c in range(NCORES):
        out += res.results[c]["y"]
    return out.reshape(B, S, H)
